# revision 39
# baseline (speedup 1.0000x reference)
"""Trainium2 Bass kernel for a 4-head spatial MultiHeadAttention block.

Reference computation (per batch n):
    q/k/v = 1x1-conv projections of x (C=256 channels, S=48*48=2304 positions)
    per head (4 heads, d=64): attn = softmax(q^T k / 8), out = attn @ v
    out = Wo @ concat(heads) + bo + x   (residual)

Sharding across 8 NeuronCores: core c handles batch n = c//2 and head-pair
hp = c%2 (output channels [hp*128, hp*128+128) of the QKV projections, i.e.
heads {2*hp, 2*hp+1}).  Each core returns per-head UNNORMALIZED Wo partials
pA = Wo[:,chA] @ rawA and pB (256 x 2304 each) plus the softmax row-sum rows
zA/zB; the host computes sum_c(pA/zA + pB/zB) + bo + Wo@bv + x.  Host-side
normalization is exact (softmax denominators commute with Wo) and removes
the on-device reciprocal-broadcast machinery entirely.

Per-core kernel layout choices (v3):
  - PE column-rate reality (measured): 1 column/cycle @2.4GHz only with
    K=128 contraction; K=64 matmuls run at HALF rate, so zero-padded K=128
    beats "row-tiled" K=64 pairs.  fp8 DoubleRow (K=256 effective) runs at
    the same 2 bf16-matmuls-per-427ns rate but halves instruction count.
  - x, Wq, Wk, Wv are fp8(e4m3); weights pre-scaled by 16 on the host so
    their sigma=1/16 values sit in fp8's normal range.  QKV projections
    contract 256 channels as one fp8 DoubleRow matmul per chunk.
  - scores: q_sb holds 16q bf16 (d on partitions, head A rows 0-63, head B
    64-127); K stored zero-padded per head (kz0/kz1) so every scores matmul
    contracts the full 128 partitions at full rate.
  - exp on ScalarE: exp(score*2^-11 - 2) written directly as fp8 e4m3.  The
    -2 bias centers the range (max ~54 << 240); it cancels in the host-side
    normalization.  A dummy activation in the prologue prefetches the exp
    table set off the critical path.
  - attn@V: VT fp8 with a ones-column per head (rowsums for free), laid out
    (128t, tt, 144) so per-head t-tile-PAIR slices have a 144B stride (16B
    aligned); attn@V contracts pairs via fp8 DoubleRow (one matmul per 2
    t-tiles).
  - raw attention outputs stay on partitions 0-63 for BOTH heads (attnA/
    attnB tiles, rows 64-127 zeroed once); Wo uses per-head zero-padded
    wotA/wotB so all 4 Wo matmuls per chunk contract K=128 at full rate
    with no partition-shift DMA.
  - v-bias folds into the host epilogue (softmax rows sum to 1).
  - PSUM: scA/scB (2 banks each, single-buffered; A/B exp alternation gives
    each a full exp-slot of slack), otA/otB (1 bank each), proj/wo pool
    (1 bank x 2).  Total exactly 8 banks.
  - schedule: minimal prologue (K/Q chunk 0 only), remaining K/Q/V
    projections woven one bundle per exp-group through chunk 0, attn@V of
    group g emitted after scores of g+1 (software pipeline), per-chunk Wo
    deferred to the next chunk's mid-point, and a parallel two-engine drain
    on the final chunk.  Steady state is ScalarE-bound at ~97% duty; the
    remaining span is ~15us of fixed NEFF preamble/epilogue.
"""

import numpy as np

import concourse.bass as bass
import concourse.mybir as mybir
import concourse.tile as tile
from concourse import bacc
from concourse.bass_utils import run_bass_kernel_spmd

C = 256          # channels
S = 2304         # spatial positions (48*48)
HD = 64          # head dim
P = 128          # partitions
TT = S // P      # 18 t-tiles of 128
NG = TT // 2     # 9 t-tile pairs (DoubleRow attn@V granularity)
SCALE = 1.0 / 2048.0   # (1/sqrt(64)) / (16*16) weight prescale
EXP_BIAS = -2.0
F32 = mybir.dt.float32
BF16 = mybir.dt.bfloat16
FP8 = mybir.dt.float8e4
DR = mybir.MatmulPerfMode.DoubleRow

S_CHUNKS = [(0, 512), (512, 512), (1024, 512), (1536, 512), (2048, 256)]


def _body(tc):
    nc = tc.nc
    t_x = nc.dram_tensor("x8", [P, 2 * S], FP8, kind="ExternalInput").ap()
    t_wq = nc.dram_tensor("wq8", [P, 2 * P], FP8, kind="ExternalInput").ap()
    t_wk = nc.dram_tensor("wk8", [P, 2 * P], FP8, kind="ExternalInput").ap()
    t_wv = nc.dram_tensor("wv8", [P, 2 * P], FP8, kind="ExternalInput").ap()
    t_wo = nc.dram_tensor("wop", [2 * P, C], BF16, kind="ExternalInput").ap()
    t_bq = nc.dram_tensor("bq", [P, 1], F32, kind="ExternalInput").ap()
    t_bk = nc.dram_tensor("bk", [P, 1], F32, kind="ExternalInput").ap()
    t_out = nc.dram_tensor("out", [2 * C, S], BF16, kind="ExternalOutput").ap()
    t_z = nc.dram_tensor("zrow", [2, S], F32, kind="ExternalOutput").ap()

    t_x3 = t_x.rearrange("p (a s) -> p a s", a=2)

    singles = tc.alloc_tile_pool(name="singles", bufs=1)
    x8 = singles.tile([P, 2, S], FP8)
    q_sb = singles.tile([P, S], BF16)
    kz0 = singles.tile([P, S], BF16)          # head A k rows 0-63, zeros 64-127
    kz1 = singles.tile([P, S], BF16)          # zeros 0-63, head B k rows 64-127
    vt_sb = singles.tile([P, TT, 144], FP8)   # per tt: [vA(64)|1|pad7|vB(64)|1|pad7]
    wq_sb = singles.tile([P, 2, P], FP8)
    wk_sb = singles.tile([P, 2, P], FP8)
    wv_sb = singles.tile([P, 2, P], FP8)
    woA_sb = singles.tile([P, C], BF16)       # Wo cols of head A on rows 0-63, 0 pad
    woB_sb = singles.tile([P, C], BF16)       # Wo cols of head B on rows 0-63, 0 pad
    attnA = singles.tile([P, S], BF16)        # raw exp@V head A rows 0-63; 0 pad
    attnB = singles.tile([P, S], BF16)
    bq_sb = singles.tile([P, 1], F32)
    bk_sb = singles.tile([P, 1], F32)
    ebias_sb = singles.tile([P, 1], F32)
    escr = singles.tile([P, 1], F32)
    zsumA = singles.tile([HD + 1, S], F32)    # row 64 = head A softmax denominators
    zsumB = singles.tile([HD + 1, S], F32)

    # ---- input DMAs: what K/Q-chunk-0 needs first, then the rest ----
    nc.gpsimd.dma_start(out=x8[:, :, 0:512], in_=t_x3[:, :, 0:512])
    nc.sync.dma_start(out=wk_sb, in_=t_wk.rearrange("p (a d) -> p a d", a=2))
    nc.sync.dma_start(out=bk_sb, in_=t_bk)
    nc.sync.dma_start(out=wq_sb, in_=t_wq.rearrange("p (a d) -> p a d", a=2))
    nc.sync.dma_start(out=bq_sb, in_=t_bq)
    for s0, sw in S_CHUNKS[1:]:
        nc.sync.dma_start(out=x8[:, :, s0:s0 + sw], in_=t_x3[:, :, s0:s0 + sw])
    nc.sync.dma_start(out=wv_sb, in_=t_wv.rearrange("p (a d) -> p a d", a=2))
    nc.sync.dma_start(out=woA_sb, in_=t_wo[0:P, :])
    nc.sync.dma_start(out=woB_sb, in_=t_wo[P:2 * P, :])
    nc.vector.memset(ebias_sb, EXP_BIAS)
    # prefetch the exp table set while DMAs run
    nc.scalar.activation(escr, ebias_sb, mybir.ActivationFunctionType.Exp,
                         bias=ebias_sb, scale=SCALE)
    # ones-columns (64/136) of vt survive the per-tile evictions, which
    # overwrite only cols 0-63 and 72-135.  Big memsets go to the
    # otherwise-idle gpsimd engine; kz dead halves are zeroed per chunk
    # inside k_chunk (chunk 0 on the faster DVE) so the pipeline can start
    # right after chunk 0's projections.
    nc.gpsimd.memset(vt_sb[:, :, :], 1.0)
    nc.gpsimd.memset(attnA[HD:P, :], 0.0)
    nc.gpsimd.memset(attnB[HD:P, :], 0.0)

    ps_sc = tc.alloc_tile_pool(name="ps_sc", bufs=1, space="PSUM")
    ps_ot = tc.alloc_tile_pool(name="ps_ot", bufs=1, space="PSUM")
    ps_w = tc.alloc_tile_pool(name="ps_w", bufs=2, space="PSUM")
    ex_pool = tc.alloc_tile_pool(name="ex_sb", bufs=3)
    wo_out = tc.alloc_tile_pool(name="wo_out", bufs=4)

    def k_chunk(s0, sw):
        meng = nc.vector if s0 == 0 else nc.gpsimd
        meng.memset(kz0[HD:P, s0:s0 + sw], 0.0)
        meng.memset(kz1[0:HD, s0:s0 + sw], 0.0)
        psn = ps_w.tile([P, 512], F32, tag="pw", name="kps")[:, :sw]
        nc.tensor.matmul(psn, wk_sb, x8[:, :, s0:s0 + sw],
                         start=True, stop=True, perf_mode=DR)
        nc.vector.tensor_scalar_add(kz0[0:HD, s0:s0 + sw], psn[0:HD, :],
                                    bk_sb[0:HD, :])
        nc.vector.tensor_scalar_add(kz1[HD:P, s0:s0 + sw], psn[HD:P, :],
                                    bk_sb[HD:P, :])

    def q_chunk(s0, sw):
        psn = ps_w.tile([P, 512], F32, tag="pw", name="qps")[:, :sw]
        nc.tensor.matmul(psn, wq_sb, x8[:, :, s0:s0 + sw],
                         start=True, stop=True, perf_mode=DR)
        nc.vector.tensor_scalar_add(q_sb[:, s0:s0 + sw], psn, bq_sb)

    def vt_batch(tt0, ntt):
        psn = ps_w.tile([P, 4, P], F32, tag="pw", name="vps")[:, :ntt, :]
        for i in range(ntt):
            tt = tt0 + i
            nc.tensor.matmul(psn[:, i, :], x8[:, :, tt * P:(tt + 1) * P], wv_sb,
                             start=True, stop=True, perf_mode=DR)
        # rows t, cols d: head A cols 0-63 -> vt col 0, head B 64-127 -> col 72
        nc.vector.tensor_scalar_mul(vt_sb[:, tt0:tt0 + ntt, 0:HD],
                                    psn[:, :, 0:HD], 1.0 / 16.0)
        nc.vector.tensor_scalar_mul(vt_sb[:, tt0:tt0 + ntt, 72:72 + HD],
                                    psn[:, :, HD:P], 1.0 / 16.0)

    cur_ot = [None, None]

    def emit_av(h, g, ex, sw):
        if g == 0:
            cur_ot[h] = ps_ot.tile([65, 512], F32, tag=f"ot{h}",
                                   name=f"ot{h}")[:, :sw]
        nc.tensor.matmul(cur_ot[h], vt_sb[:, 2 * g:2 * g + 2, 72 * h:72 * h + 65],
                         ex[:, :, :sw], start=(g == 0), stop=(g == NG - 1),
                         perf_mode=DR)

    def emit_norm(h, s0, sw, last=False):
        # raw attention values to SBUF (rows 0-63) + rowsum row accumulated
        # in zsum (stored to DRAM once per head at the end).  On the final
        # chunk head A drains on DVE during the final exp and head B on the
        # by-then-idle ScalarE, so the two tail chains run in parallel.
        ot = cur_ot[h]
        attn = attnA if h == 0 else attnB
        zsum = zsumA if h == 0 else zsumB
        if last and h == 1:
            nc.scalar.copy(attn[0:HD, s0:s0 + sw], ot[0:HD, :])
            nc.scalar.copy(zsum[HD:HD + 1, s0:s0 + sw], ot[HD:HD + 1, :])
        else:
            nc.vector.tensor_copy(attn[0:HD, s0:s0 + sw], ot[0:HD, :])
            nc.vector.tensor_copy(zsum[HD:HD + 1, s0:s0 + sw], ot[HD:HD + 1, :])
        if last:
            # single batched rowsum store per head, issued at the tail
            zeng = nc.scalar if h == 1 else nc.sync
            zeng.dma_start(out=t_z[h:h + 1, :], in_=zsum[HD:HD + 1, :])

    # t_out rows are (2h+half)*128 + p; [half][p][h][s] view matches the
    # per-half ob tile layout so one DMA covers both heads
    t_out4 = t_out.rearrange("(h x p) s -> x p h s", h=2, x=2)

    def wo_head(h, s0, sw):
        wot = woA_sb if h == 0 else woB_sb
        attn = attnA if h == 0 else attnB
        for half in range(2):
            psn = ps_w.tile([P, 512], F32, tag="pw", name="wops")[:, :sw]
            nc.tensor.matmul(psn, wot[:, half * P:(half + 1) * P],
                             attn[:, s0:s0 + sw], start=True, stop=True)
            ob = wo_out.tile([P, 512], BF16, tag="obh", name="ob")[:, :sw]
            nc.vector.tensor_copy(ob, psn)
            eng = nc.sync if half == 0 else nc.gpsimd
            eng.dma_start(out=t_out[(2 * h + half) * P:(2 * h + half + 1) * P,
                                    s0:s0 + sw], in_=ob)

    def wo_chunk(s0, sw):
        ob2 = [wo_out.tile([P, 2, 512], BF16, tag=f"ob{half}",
                           name=f"ob{half}")[:, :, :sw] for half in range(2)]
        for h in range(2):
            wot = woA_sb if h == 0 else woB_sb
            attn = attnA if h == 0 else attnB
            for half in range(2):
                psn = ps_w.tile([P, 512], F32, tag="pw", name="wops")[:, :sw]
                nc.tensor.matmul(psn, wot[:, half * P:(half + 1) * P],
                                 attn[:, s0:s0 + sw], start=True, stop=True)
                nc.vector.tensor_copy(ob2[half][:, h, :], psn)
        for half in range(2):
            eng = nc.sync if half == 0 else nc.gpsimd
            eng.dma_start(out=t_out4[half, :, :, s0:s0 + sw], in_=ob2[half])

    # ---- minimal prologue, ordered by what gates the FIRST exp (head A):
    # kz0-add and q-add lead; head B's kz1 work follows them on the DVE queue
    s0p, swp = S_CHUNKS[0]
    nc.vector.memset(kz0[HD:P, s0p:s0p + swp], 0.0)
    nc.vector.memset(kz1[0:HD, s0p:s0p + swp], 0.0)
    kps0 = ps_w.tile([P, 512], F32, tag="pw", name="kps0")[:, :swp]
    nc.tensor.matmul(kps0, wk_sb, x8[:, :, s0p:s0p + swp],
                     start=True, stop=True, perf_mode=DR)
    # all chunk-0 bias adds ride the otherwise-idle ScalarE (Identity shares
    # the exp table set) - the DVE queue never gates the first scores
    nc.scalar.add(kz0[0:HD, s0p:s0p + swp], kps0[0:HD, :], bk_sb[0:HD, :])
    qps0 = ps_w.tile([P, 512], F32, tag="pw", name="qps0")[:, :swp]
    nc.tensor.matmul(qps0, wq_sb, x8[:, :, s0p:s0p + swp],
                     start=True, stop=True, perf_mode=DR)
    nc.scalar.add(q_sb[:, s0p:s0p + swp], qps0, bq_sb)
    nc.scalar.add(kz1[HD:P, s0p:s0p + swp], kps0[HD:P, :],
                  bk_sb[HD:P, :])

    # startup weave through chunk 0 (one bundle per g): remaining K chunks
    # just ahead of the scores that read them, V-tile batches just ahead of
    # the attn@V that reads them, remaining Q chunks before chunk 1
    def KC(c):
        return lambda: k_chunk(*S_CHUNKS[c])

    def VT(b):
        return lambda: vt_batch(4 * b, min(4, TT - 4 * b))

    def QC(c):
        return lambda: q_chunk(*S_CHUNKS[c])

    # each bundle lands just ahead of its consumer: VT(b) is read by the
    # attn@V pair drained at g=2b+1, K chunk c by scores at g=2c, Q chunk c
    # by chunk c's first scores
    weave = [[KC(1)], [VT(0), KC(2)], [VT(1), KC(3)], [KC(4)], [VT(2)],
             [QC(1)], [VT(3)], [QC(2)], [VT(4)], [QC(3)], [QC(4)]]

    pend = []        # (h, g, ex, s0, sw): exp batches whose attn@V is pending
    wo_q = []        # (s0, sw) chunks whose Wo projection is pending
    nchunks = len(S_CHUNKS)

    def drain_pend():
        # attn@V for every already-exp'd batch; when a head's chunk
        # completes (g == NG-1), also snapshot its output + rowsums
        while pend:
            h, g, ex, ps0, psw = pend.pop(0)
            emit_av(h, g, ex, psw)
            if g == NG - 1:
                emit_norm(h, ps0, psw)
                if h == 1:
                    wo_q.append((ps0, psw))

    for ci, (s0, sw) in enumerate(S_CHUNKS):
        last = ci == nchunks - 1
        for g in range(NG):
            scs = []
            for h in range(2):
                sc = ps_sc.tile([P, 2, 512], F32, tag=f"sc{h}",
                                name=f"sc{h}")[:, :, :sw]
                scs.append(sc)
            for h in range(2):
                # head A's pair first: at chunk boundaries head B's buffer
                # frees one exp later, and this order keeps expA ungated
                kz = kz0 if h == 0 else kz1
                for j in range(2):
                    tt = 2 * g + j
                    nc.tensor.matmul(scs[h][:, j, :],
                                     kz[:, tt * P:(tt + 1) * P],
                                     q_sb[:, s0:s0 + sw],
                                     start=True, stop=True)
            if weave:
                for unit in weave.pop(0):
                    unit()
            drain_pend()
            if wo_q and g == 4:
                wo_chunk(*wo_q.pop(0))
            for h in range(2):
                ex = ex_pool.tile([P, 2, 512], FP8, tag=f"ex{h}",
                                  name=f"ex{h}")[:, :, :sw]
                nc.scalar.activation(ex, scs[h],
                                     mybir.ActivationFunctionType.Exp,
                                     bias=ebias_sb, scale=SCALE)
                pend.append((h, g, ex, s0, sw))
        if last:
            while pend:
                h, g, exx, ps0, psw = pend.pop(0)
                emit_av(h, g, exx, psw)
                emit_norm(h, ps0, psw, last=True)
                wo_head(h, ps0, psw)
    for wq_item in wo_q:
        wo_chunk(*wq_item)

    wo_out.release()
    ex_pool.release()
    ps_w.release()
    ps_ot.release()
    ps_sc.release()
    singles.release()


_NC_CACHE = {}


def build_nc():
    if "nc" not in _NC_CACHE:
        nc = bacc.Bacc("TRN2", target_bir_lowering=False, debug=False, num_devices=8)
        with tile.TileContext(nc) as tc:
            _body(tc)
        nc.compile()
        _NC_CACHE["nc"] = nc
    return _NC_CACHE["nc"]


def make_in_maps(x, Wq, bq, Wk, bk, Wv, bv, Wo, bo):
    import ml_dtypes
    f8 = ml_dtypes.float8_e4m3    # TRN-style e4m3 (max normal 240)
    bf16 = ml_dtypes.bfloat16
    N = x.shape[0]
    xf = np.asarray(x, np.float32).reshape(N, 2, P, S)
    x8 = np.ascontiguousarray(
        np.clip(xf, -240, 240).astype(f8).transpose(0, 2, 1, 3).reshape(N, P, 2 * S))

    def w8(Wm, ch):
        Wc = np.asarray(Wm, np.float32)[ch] * 16.0           # (128, 256)
        WcT = Wc.T.reshape(2, P, P).transpose(1, 0, 2)       # [cin_p][half][cout]
        return np.ascontiguousarray(
            np.clip(WcT, -240, 240).astype(f8).reshape(P, 2 * P))

    in_maps = []
    for c in range(8):
        n, hp = c // 2, c % 2
        ch = slice(hp * P, (hp + 1) * P)
        wot = np.asarray(Wo, np.float32)[:, ch].T            # (128, 256)
        wop = np.zeros((2 * P, C), np.float32)
        wop[0:HD] = wot[0:HD]          # head A channels on rows 0-63
        wop[P:P + HD] = wot[HD:P]      # head B channels on rows 0-63 of block 2
        in_maps.append({
            "x8": x8[n],
            "wq8": w8(Wq, ch),
            "wk8": w8(Wk, ch),
            "wv8": w8(Wv, ch),
            "wop": np.ascontiguousarray(wop.astype(bf16)),
            "bq": np.ascontiguousarray(16.0 * np.asarray(bq, np.float32)[ch].reshape(P, 1)),
            "bk": np.ascontiguousarray(16.0 * np.asarray(bk, np.float32)[ch].reshape(P, 1)),
        })
    return in_maps


def run(inputs, **kwargs):
    """Run on 8 cores; returns (full output, BassKernelResults)."""
    nc = build_nc()
    in_maps = make_in_maps(**inputs)
    res = run_bass_kernel_spmd(nc, in_maps, core_ids=list(range(8)), **kwargs)
    x = np.asarray(inputs["x"], np.float32)
    Wo = np.asarray(inputs["Wo"], np.float32)
    bv = np.asarray(inputs["bv"], np.float32)
    bo = np.asarray(inputs["bo"], np.float32)
    # v-bias folds to a constant channel offset: softmax rows sum to 1
    const = (Wo @ bv + bo)[:, None]
    N, _, H, W = x.shape
    out = np.empty((N, C, S), np.float32)
    for n in range(N):
        acc = x[n].reshape(C, S) + const
        for c in (2 * n, 2 * n + 1):
            p = res.results[c]["out"].astype(np.float32)     # (512, S)
            z = res.results[c]["zrow"].astype(np.float32)    # (2, S)
            acc = acc + p[0:C] / z[0:1] + p[C:2 * C] / z[1:2]
        out[n] = acc
    return out.reshape(N, C, H, W), res


def kernel(**inputs):
    out, _ = run(inputs)
    return out


# revision 41
# speedup vs baseline: 1.0145x; 1.0145x over previous
"""Trainium2 Bass kernel for a 4-head spatial MultiHeadAttention block.

Reference computation (per batch n):
    q/k/v = 1x1-conv projections of x (C=256 channels, S=48*48=2304 positions)
    per head (4 heads, d=64): attn = softmax(q^T k / 8), out = attn @ v
    out = Wo @ concat(heads) + bo + x   (residual)

Sharding across 8 NeuronCores: core c handles batch n = c//2 and head-pair
hp = c%2 (output channels [hp*128, hp*128+128) of the QKV projections, i.e.
heads {2*hp, 2*hp+1}).  Each core returns per-head UNNORMALIZED Wo partials
pA = Wo[:,chA] @ rawA and pB (256 x 2304 each) plus the softmax row-sum rows
zA/zB; the host computes sum_c(pA/zA + pB/zB) + bo + Wo@bv + x.  Host-side
normalization is exact (softmax denominators commute with Wo) and removes
the on-device reciprocal-broadcast machinery entirely.

Per-core kernel layout choices (v3):
  - PE column-rate reality (measured): 1 column/cycle @2.4GHz only with
    K=128 contraction; K=64 matmuls run at HALF rate, so zero-padded K=128
    beats "row-tiled" K=64 pairs.  fp8 DoubleRow (K=256 effective) runs at
    the same 2 bf16-matmuls-per-427ns rate but halves instruction count.
  - x, Wq, Wk, Wv are fp8(e4m3); weights pre-scaled by 16 on the host so
    their sigma=1/16 values sit in fp8's normal range.  QKV projections
    contract 256 channels as one fp8 DoubleRow matmul per chunk.
  - scores: q_sb holds 16q bf16 (d on partitions, head A rows 0-63, head B
    64-127); K stored zero-padded per head (kz0/kz1) so every scores matmul
    contracts the full 128 partitions at full rate.
  - exp on ScalarE: exp(score*2^-11 - 2) written directly as fp8 e4m3.  The
    -2 bias centers the range (max ~54 << 240); it cancels in the host-side
    normalization.  A dummy activation in the prologue prefetches the exp
    table set off the critical path.
  - attn@V: VT fp8 with a ones-column per head (rowsums for free), laid out
    (128t, tt, 144) so per-head t-tile-PAIR slices have a 144B stride (16B
    aligned); attn@V contracts pairs via fp8 DoubleRow (one matmul per 2
    t-tiles).
  - raw attention outputs stay on partitions 0-63 for BOTH heads (attnA/
    attnB tiles, rows 64-127 zeroed once); Wo uses per-head zero-padded
    wotA/wotB so all 4 Wo matmuls per chunk contract K=128 at full rate
    with no partition-shift DMA.
  - v-bias folds into the host epilogue (softmax rows sum to 1).
  - PSUM: scA/scB (2 banks each, single-buffered; A/B exp alternation gives
    each a full exp-slot of slack), otA/otB (1 bank each), proj/wo pool
    (1 bank x 2).  Total exactly 8 banks.
  - schedule: minimal prologue (K/Q chunk 0 only), remaining K/Q/V
    projections woven one bundle per exp-group through chunk 0, attn@V of
    group g emitted after scores of g+1 (software pipeline), per-chunk Wo
    deferred to the next chunk's mid-point, and a parallel two-engine drain
    on the final chunk.  Steady state is ScalarE-bound at ~97% duty; the
    remaining span is ~15us of fixed NEFF preamble/epilogue.
"""

import numpy as np

import concourse.bass as bass
import concourse.mybir as mybir
import concourse.tile as tile
from concourse import bacc
from concourse.bass_utils import run_bass_kernel_spmd

C = 256          # channels
S = 2304         # spatial positions (48*48)
HD = 64          # head dim
P = 128          # partitions
TT = S // P      # 18 t-tiles of 128
NG = TT // 2     # 9 t-tile pairs (DoubleRow attn@V granularity)
SCALE = 1.0 / 2048.0   # (1/sqrt(64)) / (16*16) weight prescale
EXP_BIAS = -2.0
F32 = mybir.dt.float32
BF16 = mybir.dt.bfloat16
FP8 = mybir.dt.float8e4
DR = mybir.MatmulPerfMode.DoubleRow

S_CHUNKS = [(0, 512), (512, 512), (1024, 512), (1536, 512), (2048, 256)]


def _body(tc):
    nc = tc.nc
    t_x = nc.dram_tensor("x8", [P, 2 * S], FP8, kind="ExternalInput").ap()
    t_wq = nc.dram_tensor("wq8", [P, 2 * P], FP8, kind="ExternalInput").ap()
    t_wk = nc.dram_tensor("wk8", [P, 2 * P], FP8, kind="ExternalInput").ap()
    t_wv = nc.dram_tensor("wv8", [P, 2 * P], FP8, kind="ExternalInput").ap()
    t_wo = nc.dram_tensor("wop", [2 * P, C], BF16, kind="ExternalInput").ap()
    t_bq = nc.dram_tensor("bq", [P, 1], F32, kind="ExternalInput").ap()
    t_bk = nc.dram_tensor("bk", [P, 1], F32, kind="ExternalInput").ap()
    t_out = nc.dram_tensor("out", [2 * C, S], BF16, kind="ExternalOutput").ap()
    t_z = nc.dram_tensor("zrow", [2, S], F32, kind="ExternalOutput").ap()

    t_x3 = t_x.rearrange("p (a s) -> p a s", a=2)

    singles = tc.alloc_tile_pool(name="singles", bufs=1)
    x8 = singles.tile([P, 2, S], FP8)
    q_sb = singles.tile([P, S], BF16)
    kz0 = singles.tile([P, S], BF16)          # head A k rows 0-63, zeros 64-127
    kz1 = singles.tile([P, S], BF16)          # zeros 0-63, head B k rows 64-127
    vt_sb = singles.tile([P, TT, 144], FP8)   # per tt: [vA(64)|1|pad7|vB(64)|1|pad7]
    wq_sb = singles.tile([P, 2, P], FP8)
    wk_sb = singles.tile([P, 2, P], FP8)
    wv_sb = singles.tile([P, 2, P], FP8)
    woA_sb = singles.tile([P, C], BF16)       # Wo cols of head A on rows 0-63, 0 pad
    woB_sb = singles.tile([P, C], BF16)       # Wo cols of head B on rows 0-63, 0 pad
    attnA = singles.tile([P, S], BF16)        # raw exp@V head A rows 0-63; 0 pad
    attnB = singles.tile([P, S], BF16)
    bq_sb = singles.tile([P, 1], F32)
    bk_sb = singles.tile([P, 1], F32)
    ebias_sb = singles.tile([P, 1], F32)
    escr = singles.tile([P, 1], F32)
    zsumA = singles.tile([HD + 1, S], F32)    # row 64 = head A softmax denominators
    zsumB = singles.tile([HD + 1, S], F32)

    # ---- input DMAs: what K/Q-chunk-0 needs first, then the rest ----
    nc.gpsimd.dma_start(out=wk_sb, in_=t_wk.rearrange("p (a d) -> p a d", a=2))
    nc.gpsimd.dma_start(out=bk_sb, in_=t_bk)
    nc.sync.dma_start(out=x8[:, :, 0:512], in_=t_x3[:, :, 0:512])
    nc.sync.dma_start(out=wq_sb, in_=t_wq.rearrange("p (a d) -> p a d", a=2))
    nc.sync.dma_start(out=bq_sb, in_=t_bq)
    for s0, sw in S_CHUNKS[1:]:
        nc.sync.dma_start(out=x8[:, :, s0:s0 + sw], in_=t_x3[:, :, s0:s0 + sw])
    nc.sync.dma_start(out=wv_sb, in_=t_wv.rearrange("p (a d) -> p a d", a=2))
    nc.sync.dma_start(out=woA_sb, in_=t_wo[0:P, :])
    nc.sync.dma_start(out=woB_sb, in_=t_wo[P:2 * P, :])
    nc.vector.memset(ebias_sb, EXP_BIAS)
    # prefetch the exp table set while DMAs run
    nc.scalar.activation(escr, ebias_sb, mybir.ActivationFunctionType.Exp,
                         bias=ebias_sb, scale=SCALE)
    # ones-columns (64/136) of vt survive the per-tile evictions, which
    # overwrite only cols 0-63 and 72-135.  Big memsets go to the
    # otherwise-idle gpsimd engine; kz dead halves are zeroed per chunk
    # inside k_chunk (chunk 0 on the faster DVE) so the pipeline can start
    # right after chunk 0's projections.
    nc.gpsimd.memset(vt_sb[:, :, :], 1.0)
    nc.gpsimd.memset(attnA[HD:P, :], 0.0)
    nc.gpsimd.memset(attnB[HD:P, :], 0.0)

    ps_sc = tc.alloc_tile_pool(name="ps_sc", bufs=1, space="PSUM")
    ps_ot = tc.alloc_tile_pool(name="ps_ot", bufs=1, space="PSUM")
    ps_w = tc.alloc_tile_pool(name="ps_w", bufs=2, space="PSUM")
    ex_pool = tc.alloc_tile_pool(name="ex_sb", bufs=3)
    wo_out = tc.alloc_tile_pool(name="wo_out", bufs=4)

    def k_chunk(s0, sw):
        meng = nc.vector if s0 == 0 else nc.gpsimd
        meng.memset(kz0[HD:P, s0:s0 + sw], 0.0)
        meng.memset(kz1[0:HD, s0:s0 + sw], 0.0)
        psn = ps_w.tile([P, 512], F32, tag="pw", name="kps")[:, :sw]
        nc.tensor.matmul(psn, wk_sb, x8[:, :, s0:s0 + sw],
                         start=True, stop=True, perf_mode=DR)
        nc.vector.tensor_scalar_add(kz0[0:HD, s0:s0 + sw], psn[0:HD, :],
                                    bk_sb[0:HD, :])
        nc.vector.tensor_scalar_add(kz1[HD:P, s0:s0 + sw], psn[HD:P, :],
                                    bk_sb[HD:P, :])

    def q_chunk(s0, sw):
        psn = ps_w.tile([P, 512], F32, tag="pw", name="qps")[:, :sw]
        nc.tensor.matmul(psn, wq_sb, x8[:, :, s0:s0 + sw],
                         start=True, stop=True, perf_mode=DR)
        nc.vector.tensor_scalar_add(q_sb[:, s0:s0 + sw], psn, bq_sb)

    def vt_batch(tt0, ntt):
        psn = ps_w.tile([P, 4, P], F32, tag="pw", name="vps")[:, :ntt, :]
        for i in range(ntt):
            tt = tt0 + i
            nc.tensor.matmul(psn[:, i, :], x8[:, :, tt * P:(tt + 1) * P], wv_sb,
                             start=True, stop=True, perf_mode=DR)
        # rows t, cols d: head A cols 0-63 -> vt col 0, head B 64-127 -> col 72
        nc.vector.tensor_scalar_mul(vt_sb[:, tt0:tt0 + ntt, 0:HD],
                                    psn[:, :, 0:HD], 1.0 / 16.0)
        nc.vector.tensor_scalar_mul(vt_sb[:, tt0:tt0 + ntt, 72:72 + HD],
                                    psn[:, :, HD:P], 1.0 / 16.0)

    cur_ot = [None, None]

    def emit_av(h, g, exs, sw):
        if g == 0:
            cur_ot[h] = ps_ot.tile([65, 512], F32, tag=f"ot{h}",
                                   name=f"ot{h}")[:, :sw]
        nc.tensor.matmul(cur_ot[h], vt_sb[:, 2 * g:2 * g + 2, 72 * h:72 * h + 65],
                         exs, start=(g == 0), stop=(g == NG - 1),
                         perf_mode=DR)

    def emit_norm(h, s0, sw, last=False):
        # raw attention values to SBUF (rows 0-63) + rowsum row accumulated
        # in zsum (stored to DRAM once per head at the end).  On the final
        # chunk head A drains on DVE during the final exp and head B on the
        # by-then-idle ScalarE, so the two tail chains run in parallel.
        ot = cur_ot[h]
        attn = attnA if h == 0 else attnB
        zsum = zsumA if h == 0 else zsumB
        if last and h == 1:
            nc.scalar.copy(attn[0:HD, s0:s0 + sw], ot[0:HD, :])
            nc.scalar.copy(zsum[HD:HD + 1, s0:s0 + sw], ot[HD:HD + 1, :])
        else:
            nc.vector.tensor_copy(attn[0:HD, s0:s0 + sw], ot[0:HD, :])
            nc.vector.tensor_copy(zsum[HD:HD + 1, s0:s0 + sw], ot[HD:HD + 1, :])
        if last:
            # single batched rowsum store per head, issued at the tail
            zeng = nc.scalar if h == 1 else nc.sync
            zeng.dma_start(out=t_z[h:h + 1, :], in_=zsum[HD:HD + 1, :])

    # t_out rows are (2h+half)*128 + p; [half][p][h][s] view matches the
    # per-half ob tile layout so one DMA covers both heads
    t_out4 = t_out.rearrange("(h x p) s -> x p h s", h=2, x=2)

    def wo_head(h, s0, sw):
        wot = woA_sb if h == 0 else woB_sb
        attn = attnA if h == 0 else attnB
        for half in range(2):
            psn = ps_w.tile([P, 512], F32, tag="pw", name="wops")[:, :sw]
            nc.tensor.matmul(psn, wot[:, half * P:(half + 1) * P],
                             attn[:, s0:s0 + sw], start=True, stop=True)
            ob = wo_out.tile([P, 512], BF16, tag="obh", name="ob")[:, :sw]
            nc.vector.tensor_copy(ob, psn)
            eng = nc.sync if half == 0 else nc.gpsimd
            eng.dma_start(out=t_out[(2 * h + half) * P:(2 * h + half + 1) * P,
                                    s0:s0 + sw], in_=ob)

    def wo_chunk(s0, sw):
        ob2 = [wo_out.tile([P, 2, 512], BF16, tag=f"ob{half}",
                           name=f"ob{half}")[:, :, :sw] for half in range(2)]
        for h in range(2):
            wot = woA_sb if h == 0 else woB_sb
            attn = attnA if h == 0 else attnB
            for half in range(2):
                psn = ps_w.tile([P, 512], F32, tag="pw", name="wops")[:, :sw]
                nc.tensor.matmul(psn, wot[:, half * P:(half + 1) * P],
                                 attn[:, s0:s0 + sw], start=True, stop=True)
                nc.vector.tensor_copy(ob2[half][:, h, :], psn)
        for half in range(2):
            eng = nc.sync if half == 0 else nc.gpsimd
            eng.dma_start(out=t_out4[half, :, :, s0:s0 + sw], in_=ob2[half])

    # ---- minimal prologue, ordered by what gates the FIRST exp (head A):
    # kz0-add and q-add lead; head B's kz1 work follows them on the DVE queue
    s0p, swp = S_CHUNKS[0]
    nc.vector.memset(kz0[HD:P, s0p:s0p + swp], 0.0)
    nc.vector.memset(kz1[0:HD, s0p:s0p + swp], 0.0)
    kps0 = ps_w.tile([P, 512], F32, tag="pw", name="kps0")[:, :swp]
    nc.tensor.matmul(kps0, wk_sb, x8[:, :, s0p:s0p + swp],
                     start=True, stop=True, perf_mode=DR)
    # all chunk-0 bias adds ride the otherwise-idle ScalarE (Identity shares
    # the exp table set) - the DVE queue never gates the first scores
    nc.scalar.add(kz0[0:HD, s0p:s0p + swp], kps0[0:HD, :], bk_sb[0:HD, :])
    qps0 = ps_w.tile([P, 512], F32, tag="pw", name="qps0")[:, :swp]
    nc.tensor.matmul(qps0, wq_sb, x8[:, :, s0p:s0p + swp],
                     start=True, stop=True, perf_mode=DR)
    nc.scalar.add(q_sb[:, s0p:s0p + swp], qps0, bq_sb)
    nc.scalar.add(kz1[HD:P, s0p:s0p + swp], kps0[HD:P, :],
                  bk_sb[HD:P, :])

    # startup weave through chunk 0 (one bundle per g): remaining K chunks
    # just ahead of the scores that read them, V-tile batches just ahead of
    # the attn@V that reads them, remaining Q chunks before chunk 1
    def KC(c):
        return lambda: k_chunk(*S_CHUNKS[c])

    def VT(b):
        return lambda: vt_batch(4 * b, min(4, TT - 4 * b))

    def QC(c):
        return lambda: q_chunk(*S_CHUNKS[c])

    # each bundle lands just ahead of its consumer: VT(b) is read by the
    # attn@V pair drained at g=2b+1, K chunk c by scores at g=2c, Q chunk c
    # by chunk c's first scores
    weave = [[KC(1)], [VT(0), KC(2)], [VT(1), KC(3)], [KC(4)], [VT(2)],
             [QC(1)], [VT(3)], [QC(2)], [VT(4)], [QC(3)], [QC(4)]]

    pend = []        # (h, g, ex, s0, sw): exp batches whose attn@V is pending
    wo_q = []        # (s0, sw) chunks whose Wo projection is pending
    nchunks = len(S_CHUNKS)

    def drain_pend():
        # attn@V for every already-exp'd batch; when a head's chunk
        # completes (g == NG-1), also snapshot its output + rowsums
        while pend:
            h, g, ex, ps0, psw = pend.pop(0)
            emit_av(h, g, ex, psw)
            if g == NG - 1:
                emit_norm(h, ps0, psw)
                if h == 1:
                    wo_q.append((ps0, psw))

    for ci, (s0, sw) in enumerate(S_CHUNKS):
        last = ci == nchunks - 1
        if sw == 512:
            groups = [(2 * g, 2) for g in range(NG)]
            shp = [P, 2, 512]
            womark = 4
        else:
            # narrow chunk: same PSUM bytes hold 4 t-tiles per batch, so the
            # exp instruction count drops from 9 to 5 per head
            groups = [(0, 4), (4, 4), (8, 4), (12, 4), (16, 2)]
            shp = [P, 4, 256]
            womark = 2
        for gi, (tt0, ntt) in enumerate(groups):
            scs = []
            for h in range(2):
                sc = ps_sc.tile(shp, F32, tag=f"sc{h}",
                                name=f"sc{h}")[:, :ntt, :]
                scs.append(sc)
            for h in range(2):
                # head A's tiles first: at chunk boundaries head B's buffer
                # frees one exp later, and this order keeps expA ungated
                kz = kz0 if h == 0 else kz1
                for j in range(ntt):
                    tt = tt0 + j
                    nc.tensor.matmul(scs[h][:, j, :],
                                     kz[:, tt * P:(tt + 1) * P],
                                     q_sb[:, s0:s0 + sw],
                                     start=True, stop=True)
            if weave:
                for unit in weave.pop(0):
                    unit()
            drain_pend()
            if wo_q and gi == womark:
                wo_chunk(*wo_q.pop(0))
            for h in range(2):
                ex = ex_pool.tile(shp, FP8, tag=f"ex{h}",
                                  name=f"ex{h}")[:, :ntt, :]
                nc.scalar.activation(ex, scs[h],
                                     mybir.ActivationFunctionType.Exp,
                                     bias=ebias_sb, scale=SCALE)
                for j2 in range(ntt // 2):
                    pend.append((h, tt0 // 2 + j2,
                                 ex[:, 2 * j2:2 * j2 + 2, :], s0, sw))
        if last:
            while pend:
                h, g, exx, ps0, psw = pend.pop(0)
                emit_av(h, g, exx, psw)
                emit_norm(h, ps0, psw, last=True)
                wo_head(h, ps0, psw)
    for wq_item in wo_q:
        wo_chunk(*wq_item)

    wo_out.release()
    ex_pool.release()
    ps_w.release()
    ps_ot.release()
    ps_sc.release()
    singles.release()


_NC_CACHE = {}


def build_nc():
    if "nc" not in _NC_CACHE:
        nc = bacc.Bacc("TRN2", target_bir_lowering=False, debug=False, num_devices=8)
        with tile.TileContext(nc) as tc:
            _body(tc)
        nc.compile()
        _NC_CACHE["nc"] = nc
    return _NC_CACHE["nc"]


def make_in_maps(x, Wq, bq, Wk, bk, Wv, bv, Wo, bo):
    import ml_dtypes
    f8 = ml_dtypes.float8_e4m3    # TRN-style e4m3 (max normal 240)
    bf16 = ml_dtypes.bfloat16
    N = x.shape[0]
    xf = np.asarray(x, np.float32).reshape(N, 2, P, S)
    x8 = np.ascontiguousarray(
        np.clip(xf, -240, 240).astype(f8).transpose(0, 2, 1, 3).reshape(N, P, 2 * S))

    def w8(Wm, ch):
        Wc = np.asarray(Wm, np.float32)[ch] * 16.0           # (128, 256)
        WcT = Wc.T.reshape(2, P, P).transpose(1, 0, 2)       # [cin_p][half][cout]
        return np.ascontiguousarray(
            np.clip(WcT, -240, 240).astype(f8).reshape(P, 2 * P))

    in_maps = []
    for c in range(8):
        n, hp = c // 2, c % 2
        ch = slice(hp * P, (hp + 1) * P)
        wot = np.asarray(Wo, np.float32)[:, ch].T            # (128, 256)
        wop = np.zeros((2 * P, C), np.float32)
        wop[0:HD] = wot[0:HD]          # head A channels on rows 0-63
        wop[P:P + HD] = wot[HD:P]      # head B channels on rows 0-63 of block 2
        in_maps.append({
            "x8": x8[n],
            "wq8": w8(Wq, ch),
            "wk8": w8(Wk, ch),
            "wv8": w8(Wv, ch),
            "wop": np.ascontiguousarray(wop.astype(bf16)),
            "bq": np.ascontiguousarray(16.0 * np.asarray(bq, np.float32)[ch].reshape(P, 1)),
            "bk": np.ascontiguousarray(16.0 * np.asarray(bk, np.float32)[ch].reshape(P, 1)),
        })
    return in_maps


def run(inputs, **kwargs):
    """Run on 8 cores; returns (full output, BassKernelResults)."""
    nc = build_nc()
    in_maps = make_in_maps(**inputs)
    res = run_bass_kernel_spmd(nc, in_maps, core_ids=list(range(8)), **kwargs)
    x = np.asarray(inputs["x"], np.float32)
    Wo = np.asarray(inputs["Wo"], np.float32)
    bv = np.asarray(inputs["bv"], np.float32)
    bo = np.asarray(inputs["bo"], np.float32)
    # v-bias folds to a constant channel offset: softmax rows sum to 1
    const = (Wo @ bv + bo)[:, None]
    N, _, H, W = x.shape
    out = np.empty((N, C, S), np.float32)
    for n in range(N):
        acc = x[n].reshape(C, S) + const
        for c in (2 * n, 2 * n + 1):
            p = res.results[c]["out"].astype(np.float32)     # (512, S)
            z = res.results[c]["zrow"].astype(np.float32)    # (2, S)
            acc = acc + p[0:C] / z[0:1] + p[C:2 * C] / z[1:2]
        out[n] = acc
    return out.reshape(N, C, H, W), res


def kernel(**inputs):
    out, _ = run(inputs)
    return out
